# revision 18
# baseline (speedup 1.0000x reference)
"""Trainium2 Bass kernel: BiLSTM + CRF negative log-likelihood (mean over batch).

Contract: kernel(**inputs) takes the FULL unsharded inputs (B=64, S=512) and
returns the scalar fp32 NLL.  Internally the batch is sharded 8 ways across
8 NeuronCores (8 sequences per core); the embedding table is replicated and
gathered on-device via indirect DMA.  Each core computes the per-sequence
log-likelihood for its 8 sequences; the host averages the 64 values.

Mask is assumed all-ones (as produced by the problem's setup_inputs).

Per-core layout choices:
 - token column index = t*BL + b (t-major), BL = 8 sequences per core
 - LSTM state feature-on-partition: h, c are [128, BL]
 - gate order re-packed (i,f,o,g) so one sigmoid covers i,f,o
 - CRF denominator: exp-space chunked parallel scan over 16 chunks
   (slots (chunk,b) = 128 partitions in the combine stage), with the 9x9
   exp(trans) as the PE stationary during the scan.
"""
import numpy as np

import concourse.bacc as bacc
import concourse.bass as bass
import concourse.mybir as mybir
import concourse.tile as tile
from concourse.bass_utils import run_bass_kernel_spmd

AF = mybir.ActivationFunctionType
ALU = mybir.AluOpType
AX = mybir.AxisListType
F32 = mybir.dt.float32
BF16 = mybir.dt.bfloat16
I32 = mybir.dt.int32

V, E, EP = 100000, 300, 384
HD, NG = 128, 4
NT = 9
NCORES = 8
CH = 16
LNS = -2.0

DIRS = ("f", "b")


def build(S, BL):
    NTOK = S * BL
    TPT = 128 // BL
    NTT = NTOK // 128
    CL = S // CH
    GW = NG * BL                 # 32
    SLOTW = BL * NT              # 72

    nc = bacc.Bacc(None, target_bir_lowering=False, debug=False)

    emb = nc.dram_tensor("emb", [V, E], F32, kind="ExternalInput")
    widx = nc.dram_tensor("widx", [NTOK, 1], I32, kind="ExternalInput")
    tagsF = nc.dram_tensor("tagsF", [1, NTOK], F32, kind="ExternalInput")
    wihT = {d: nc.dram_tensor(f"wihT_{d}", [EP, NG * HD], F32, kind="ExternalInput")
            for d in DIRS}
    whhT = {d: nc.dram_tensor(f"whhT_{d}", [HD, NG * HD], F32, kind="ExternalInput")
            for d in DIRS}
    biasD = {d: nc.dram_tensor(f"bias_{d}", [HD, NG], F32, kind="ExternalInput")
             for d in DIRS}
    woT = nc.dram_tensor("woT", [2 * HD, NT], F32, kind="ExternalInput")
    bout = nc.dram_tensor("bout", [NT, 1], F32, kind="ExternalInput")
    transD = nc.dram_tensor("trans", [NT, NT], F32, kind="ExternalInput")
    transTD = nc.dram_tensor("transT", [NT, NT], F32, kind="ExternalInput")
    startAdjD = nc.dram_tensor("startadj", [NT, 1], F32, kind="ExternalInput")
    start9D = nc.dram_tensor("start9", [NT, 1], F32, kind="ExternalInput")
    end9D = nc.dram_tensor("end9", [NT, 1], F32, kind="ExternalInput")
    iotaD = nc.dram_tensor("iota9", [NT, 1], F32, kind="ExternalInput")
    ones9D = nc.dram_tensor("ones9", [NT, 1], F32, kind="ExternalInput")
    identD = nc.dram_tensor("ident", [128, 128], F32, kind="ExternalInput")
    mtinitD = nc.dram_tensor("mtinit", [NT, CH * SLOTW], F32, kind="ExternalInput")
    llhD = nc.dram_tensor("llh", [1, BL], F32, kind="ExternalOutput")

    with tile.TileContext(nc) as tc:
        # ---------------- persistent tiles ----------------
        pers_cm = tc.tile_pool(name="pers", bufs=1)
        pers = pers_cm.__enter__()
        H = {d: pers.tile([128, NTOK], BF16, tag=f"H{d}", name=f"H{d}") for d in DIRS}
        whh_sb = {}
        bias_sb = {}
        for d in DIRS:
            whhf = pers.tile([HD, NG * HD], F32, tag=f"whhf{d}", name=f"whhf{d}")
            nc.sync.dma_start(whhf[:], whhT[d][:])
            whh_sb[d] = pers.tile([HD, NG * HD], BF16, tag=f"whh{d}", name=f"whh{d}")
            nc.vector.tensor_copy(out=whh_sb[d][:], in_=whhf[:])
            bias_sb[d] = pers.tile([HD, NG], F32, tag=f"bias{d}", name=f"bias{d}")
            nc.sync.dma_start(bias_sb[d][:], biasD[d][:])
        ident_sb = pers.tile([128, 128], F32, tag="ident", name="ident")
        nc.sync.dma_start(ident_sb[:], identD[:])
        ident_bf = pers.tile([128, 128], BF16, tag="identbf", name="identbf")
        nc.vector.tensor_copy(out=ident_bf[:], in_=ident_sb[:])
        c_st = {d: pers.tile([128, BL], F32, tag=f"c{d}", name=f"c{d}") for d in DIRS}
        z8 = pers.tile([128, BL], BF16, tag="z8", name="z8")
        nc.vector.memset(z8[:], 0.0)
        for d in DIRS:
            nc.vector.memset(c_st[d][:], 0.0)

        # ---------------- input projections into Gin ----------------
        ging_cm = tc.tile_pool(name="gin", bufs=1)
        ging = ging_cm.__enter__()
        gin = {d: ging.tile([128, S * GW], BF16, tag=f"gin{d}", name=f"gin{d}") for d in DIRS}
        ginv = {d: gin[d][:].rearrange("p (t x) -> p t x", x=GW) for d in DIRS}

        with (
            tc.tile_pool(name="pA", bufs=3) as pA,
            tc.tile_pool(name="pAw", bufs=1) as pAw,
            tc.tile_pool(name="ppA", bufs=2, space="PSUM") as ppA,
            tc.tile_pool(name="pB", bufs=3) as pB,
            tc.tile_pool(name="ppB", bufs=2, space="PSUM") as ppB,
        ):
            wih_sb = {d: [] for d in DIRS}
            for d in DIRS:
                for k in range(3):
                    wtf = pAw.tile([128, NG * HD], F32, tag=f"wihf{d}{k}", name=f"wihf{d}{k}")
                    nc.sync.dma_start(wtf[:], wihT[d][k * 128:(k + 1) * 128, :])
                    wt = pAw.tile([128, NG * HD], BF16, tag=f"wih{d}{k}", name=f"wih{d}{k}")
                    nc.vector.tensor_copy(out=wt[:], in_=wtf[:])
                    wih_sb[d].append(wt)
            tporder = []
            for i in range((NTT + 1) // 2):
                tporder.append(i)
                if NTT - 1 - i > i:
                    tporder.append(NTT - 1 - i)
            for tp in tporder:
                idx = pA.tile([128, 1], I32, tag="idx", name="idx")
                nc.sync.dma_start(idx[:], widx[tp * 128:(tp + 1) * 128, :])
                xg = pA.tile([128, EP], F32, tag="xg", name="xg")
                nc.vector.memset(xg[:, E:EP], 0.0)
                nc.gpsimd.indirect_dma_start(
                    out=xg[:, 0:E], out_offset=None, in_=emb[:],
                    in_offset=bass.IndirectOffsetOnAxis(ap=idx[:, 0:1], axis=0),
                )
                xt = []
                for k in range(3):
                    pt = ppA.tile([128, 128], F32, tag="pt", name="pt")
                    nc.tensor.transpose(pt[:], xg[:, k * 128:(k + 1) * 128],
                                        ident_sb[:])
                    xk = pA.tile([128, 128], BF16, tag=f"xt{k}", name=f"xt{k}")
                    nc.vector.tensor_copy(out=xk[:], in_=pt[:])
                    xt.append(xk)
                for d in DIRS:
                    for g in range(NG):
                        ps = ppA.tile([128, 128], F32, tag="ps", name="ps")
                        for k in range(3):
                            nc.tensor.matmul(
                                ps[:], lhsT=wih_sb[d][k][:, g * 128:(g + 1) * 128],
                                rhs=xt[k][:], start=(k == 0), stop=(k == 2))
                        dst = ginv[d][:, tp * TPT:(tp + 1) * TPT,
                                      g * BL:(g + 1) * BL]
                        src = ps[:].rearrange("p (t b) -> p t b", b=BL)
                        nc.vector.tensor_scalar_add(dst, src,
                                                    bias_sb[d][:, g:g + 1])

            # ---------------- BiLSTM recurrence ----------------
            # i,f,o weight blocks are pre-halved host-side, so one tanh
            # covers all gates: sigmoid(x) = 0.5*tanh(x/2) + 0.5.
            def lstm_step(d, t, h_prev):
                ps = ppB.tile([128, GW], F32, tag=f"ps{d}", name=f"ps{d}")
                nc.tensor.matmul(ps[:], lhsT=ident_bf[:],
                                 rhs=gin[d][:, t * GW:(t + 1) * GW],
                                 start=True, stop=False)
                for g in range(NG):
                    nc.tensor.matmul(ps[:, g * BL:(g + 1) * BL],
                                     lhsT=whh_sb[d][:, g * 128:(g + 1) * 128],
                                     rhs=h_prev, start=False,
                                     stop=(g == NG - 1))
                T = pB.tile([128, GW], F32, tag=f"T{d}", name=f"T{d}")
                nc.scalar.activation(T[:], ps[:], AF.Tanh)
                sg = pB.tile([128, 3 * BL], F32, tag=f"sg{d}", name=f"sg{d}")
                nc.vector.tensor_scalar(out=sg[:], in0=T[:, 0:3 * BL],
                                        scalar1=0.5, scalar2=0.5,
                                        op0=ALU.mult, op1=ALU.add)
                t1 = pB.tile([128, BL], F32, tag=f"t1{d}", name=f"t1{d}")
                nc.vector.tensor_tensor(out=t1[:], in0=sg[:, 0:BL],
                                        in1=T[:, 3 * BL:GW], op=ALU.mult)
                c2 = pB.tile([128, BL], F32, tag=f"c2{d}", name=f"c2{d}")
                nc.vector.tensor_tensor(out=c2[:], in0=sg[:, BL:2 * BL],
                                        in1=c_st[d][:], op=ALU.mult)
                nc.vector.tensor_tensor(out=c_st[d][:], in0=c2[:], in1=t1[:],
                                        op=ALU.add)
                tc2 = pB.tile([128, BL], F32, tag=f"tc{d}", name=f"tc{d}")
                nc.scalar.activation(tc2[:], c_st[d][:], AF.Tanh)
                nc.vector.tensor_tensor(out=H[d][:, t * BL:(t + 1) * BL],
                                        in0=sg[:, 2 * BL:3 * BL], in1=tc2[:],
                                        op=ALU.mult)

            for step in range(S):
                tf = step
                tb = S - 1 - step
                hf = z8[:] if step == 0 else H["f"][:, (tf - 1) * BL:tf * BL]
                lstm_step("f", tf, hf)
                hb = z8[:] if step == 0 else H["b"][:, (tb + 1) * BL:(tb + 2) * BL]
                lstm_step("b", tb, hb)

        ging_cm.__exit__(None, None, None)

        # ---------------- emissions + CRF ----------------
        with (
            tc.tile_pool(name="pC", bufs=1) as pC,
            tc.tile_pool(name="pCt", bufs=2) as pCt,
            tc.tile_pool(name="ppC", bufs=2, space="PSUM") as ppC,
            tc.tile_pool(name="ppD", bufs=2, space="PSUM") as ppD,
            tc.tile_pool(name="ppE", bufs=3, space="PSUM") as ppE,
        ):
            def pbig():          # [NT, 512] psum tiles (emissions/numerator/scan)
                return ppC.tile([NT, 512], F32, tag="pbig", name="pbig")

            def ptr():           # [128, NT] transpose psum tiles
                return ppD.tile([128, NT], BF16, tag="ptr", name="ptr")

            def pone():          # [1, <=NT] psum tiles
                return ppE.tile([1, NT], F32, tag="pone", name="pone")

            wo0f = pC.tile([128, NT], F32, tag="wo0f", name="wo0f")
            wo1f = pC.tile([128, NT], F32, tag="wo1f", name="wo1f")
            nc.sync.dma_start(wo0f[:], woT[0:128, :])
            nc.sync.dma_start(wo1f[:], woT[128:256, :])
            wo0 = pC.tile([128, NT], BF16, tag="wo0", name="wo0")
            wo1 = pC.tile([128, NT], BF16, tag="wo1", name="wo1")
            nc.vector.tensor_copy(out=wo0[:], in_=wo0f[:])
            nc.vector.tensor_copy(out=wo1[:], in_=wo1f[:])
            bout_sb = pC.tile([NT, 1], F32, tag="bout", name="bout")
            nc.sync.dma_start(bout_sb[:], bout[:])
            em = pC.tile([NT, NTOK], F32, tag="em", name="em")
            NCHK = (NTOK + 511) // 512
            for n in range(NCHK):
                lo, hi = n * 512, min((n + 1) * 512, NTOK)
                pe = pbig()
                nc.tensor.matmul(pe[:, 0:hi - lo], lhsT=wo0[:],
                                 rhs=H["f"][:, lo:hi], start=True, stop=False)
                nc.tensor.matmul(pe[:, 0:hi - lo], lhsT=wo1[:],
                                 rhs=H["b"][:, lo:hi], start=False, stop=True)
                nc.scalar.activation(em[:, lo:hi], pe[:, 0:hi - lo], AF.Identity,
                                     bias=bout_sb[:, 0:1])

            # --- numerator (gold path score) ---
            trT = pC.tile([NT, NT], F32, tag="trT", name="trT")
            nc.sync.dma_start(trT[:], transTD[:])
            st9 = pC.tile([NT, 1], F32, tag="st9", name="st9")
            nc.sync.dma_start(st9[:], start9D[:])
            en9 = pC.tile([NT, 1], F32, tag="en9", name="en9")
            nc.sync.dma_start(en9[:], end9D[:])
            io9 = pC.tile([NT, 1], F32, tag="io9", name="io9")
            nc.sync.dma_start(io9[:], iotaD[:])
            on9 = pC.tile([NT, 1], F32, tag="on9", name="on9")
            nc.sync.dma_start(on9[:], ones9D[:])

            tagR = pC.tile([NT, NTOK], F32, tag="tagR", name="tagR")
            tg1 = pC.tile([1, NTOK], F32, tag="tg1", name="tg1")
            nc.sync.dma_start(tg1[:], tagsF[:])
            nc.gpsimd.partition_broadcast(tagR[:], tg1[0:1, :])
            oh = pC.tile([NT, NTOK], F32, tag="oh", name="oh")
            nc.vector.tensor_tensor(out=oh[:],
                                    in0=io9[:, 0:1].to_broadcast([NT, NTOK]),
                                    in1=tagR[:], op=ALU.is_equal)
            t1f = pC.tile([NT, NTOK], F32, tag="t1f", name="t1f")
            for n in range(NCHK):
                lo, hi = n * 512, min((n + 1) * 512, NTOK)
                pn = pbig()
                nc.tensor.matmul(pn[:, 0:hi - lo], lhsT=trT[:], rhs=oh[:, lo:hi],
                                 start=True, stop=True)
                nc.vector.tensor_copy(out=t1f[:, lo:hi], in_=pn[:, 0:hi - lo])

            tmp = pC.tile([NT, NTOK], F32, tag="tmp", name="tmp")
            nc.vector.tensor_tensor(out=tmp[:], in0=em[:], in1=oh[:], op=ALU.mult)
            sc = pCt.tile([NT, BL], F32, tag="sc_em", name="sc_em")
            nc.vector.tensor_reduce(
                out=sc[:], in_=tmp[:].rearrange("p (t b) -> p b t", b=BL),
                axis=AX.X, op=ALU.add)
            tmq = pC.tile([NT, NTOK - BL], F32, tag="tmq", name="tmq")
            nc.vector.tensor_tensor(out=tmq[:], in0=oh[:, 0:NTOK - BL],
                                    in1=t1f[:, BL:NTOK], op=ALU.mult)
            sctr = pCt.tile([NT, BL], F32, tag="sc_tr", name="sc_tr")
            nc.vector.tensor_reduce(
                out=sctr[:], in_=tmq[:].rearrange("p (t b) -> p b t", b=BL),
                axis=AX.X, op=ALU.add)
            scse = pCt.tile([NT, BL], F32, tag="sc_se", name="sc_se")
            nc.vector.tensor_tensor(out=scse[:],
                                    in0=st9[:, 0:1].to_broadcast([NT, BL]),
                                    in1=oh[:, 0:BL], op=ALU.mult)
            scee = pCt.tile([NT, BL], F32, tag="sc_ee", name="sc_ee")
            nc.vector.tensor_tensor(out=scee[:],
                                    in0=en9[:, 0:1].to_broadcast([NT, BL]),
                                    in1=oh[:, NTOK - BL:NTOK], op=ALU.mult)
            nc.vector.tensor_tensor(out=sc[:], in0=sc[:], in1=sctr[:], op=ALU.add)
            nc.vector.tensor_tensor(out=scse[:], in0=scse[:], in1=scee[:],
                                    op=ALU.add)
            nc.vector.tensor_tensor(out=sc[:], in0=sc[:], in1=scse[:], op=ALU.add)
            psS = pone()
            nc.tensor.matmul(psS[:, 0:BL], lhsT=on9[:, 0:1], rhs=sc[:],
                             start=True, stop=True)
            score = pCt.tile([1, BL], F32, tag="score", name="score")
            nc.vector.tensor_copy(out=score[:], in_=psS[:, 0:BL])

            # --- denominator: exp-space chunked parallel scan ---
            trS = pC.tile([NT, NT], F32, tag="trS", name="trS")
            nc.sync.dma_start(trS[:], transD[:])
            Emat = pC.tile([NT, NT], BF16, tag="Emat", name="Emat")
            nc.scalar.activation(Emat[:], trS[:], AF.Exp)
            lnsC = pCt.tile([NT, 1], F32, tag="lnsC", name="lnsC")
            nc.vector.memset(lnsC[:], float(LNS))
            wem = pC.tile([NT, NTOK], F32, tag="wem", name="wem")
            nc.scalar.activation(wem[:], em[:], AF.Exp, bias=lnsC[:, 0:1])
            wv = wem[:].rearrange("p (c u b) -> p c u b", u=CL, b=BL)
            stA = pC.tile([NT, 1], F32, tag="stA", name="stA")
            nc.sync.dma_start(stA[:], startAdjD[:])
            Mt = pC.tile([NT, CH * SLOTW], BF16, tag="Mt", name="Mt")
            nc.gpsimd.dma_start(Mt[:], mtinitD[:])
            tmp0 = pCt.tile([NT, BL], F32, tag="tmp0", name="tmp0")
            nc.scalar.activation(tmp0[:], em[:, 0:BL], AF.Exp, bias=stA[:, 0:1])
            mtv = Mt[:].rearrange("p (s i) -> p s i", i=NT)
            for i in range(NT):
                nc.vector.tensor_copy(
                    out=mtv[:, 0:BL, i:i + 1],
                    in_=tmp0[:].rearrange("p (b o) -> p b o", o=1))

            R = [(0, 5), (5, 10), (10, CH)]
            for u in range(CL):
                for (c0, c1) in R:
                    cc0 = 1 if (u == 0 and c0 == 0) else c0
                    if cc0 >= c1:
                        continue
                    nch = c1 - cc0
                    lo, hi = cc0 * SLOTW, c1 * SLOTW
                    pm = pbig()
                    nc.tensor.matmul(pm[:, 0:hi - lo], lhsT=Emat[:],
                                     rhs=Mt[:, lo:hi], start=True, stop=True)
                    dst = Mt[:].rearrange("p (c b i) -> p c b i", b=BL, i=NT)[
                        :, cc0:c1, :, :]
                    src = pm[:, 0:hi - lo].rearrange("p (c b i) -> p c b i",
                                                     b=BL, i=NT)
                    w_in = wv[:, cc0:c1, u:u + 1, :].squeeze(2).unsqueeze(3) \
                        .broadcast_to([NT, nch, BL, NT])
                    nc.vector.tensor_tensor(out=dst, in0=src, in1=w_in,
                                            op=ALU.mult)

            # transpose Mt -> ct [slot, i*NT+j]
            ct = pC.tile([128, NT * NT], F32, tag="ct", name="ct")
            id9b = pCt.tile([NT, NT], BF16, tag="id9b", name="id9b")
            nc.vector.tensor_copy(out=id9b[:], in_=ident_sb[0:NT, 0:NT])
            mtt = Mt[:].rearrange("p (s i) -> p s i", i=NT)
            for i in range(NT):
                pt2 = ptr()
                nc.tensor.transpose(pt2[:], mtt[:, :, i:i + 1].squeeze(2),
                                    id9b[:])
                nc.vector.tensor_copy(out=ct[:, i * NT:(i + 1) * NT], in_=pt2[:])

            lgn = pC.tile([128, 1], F32, tag="lgn", name="lgn")
            mx = pCt.tile([128, 1], F32, tag="mx", name="mx")
            rc = pCt.tile([128, 1], F32, tag="rc", name="rc")
            nc.vector.tensor_reduce(out=mx[:], in_=ct[:], axis=AX.X, op=ALU.max)
            nc.vector.reciprocal(rc[:], mx[:])
            nc.vector.tensor_scalar_mul(ct[:], ct[:], rc[:, 0:1])
            nc.scalar.activation(lgn[:], mx[:], AF.Ln)

            cur, curlg, nslots = ct, lgn, 128
            for lvl in range(4):
                half = nslots // 2
                at = pCt.tile([128, NT * NT], F32, tag="at", name="at")
                bt = pCt.tile([128, NT * NT], F32, tag="bt", name="bt")
                alg = pCt.tile([128, 1], F32, tag="alg", name="alg")
                blg = pCt.tile([128, 1], F32, tag="blg", name="blg")
                for q in range(half // BL):
                    e0, e1 = (2 * q) * BL, (2 * q + 1) * BL
                    nc.sync.dma_start(at[q * BL:(q + 1) * BL, :],
                                      cur[e0:e0 + BL, :])
                    nc.sync.dma_start(bt[q * BL:(q + 1) * BL, :],
                                      cur[e1:e1 + BL, :])
                    nc.sync.dma_start(alg[q * BL:(q + 1) * BL, :],
                                      curlg[e0:e0 + BL, :])
                    nc.sync.dma_start(blg[q * BL:(q + 1) * BL, :],
                                      curlg[e1:e1 + BL, :])
                prod = pCt.tile([128, NT * NT * NT], F32, tag="prod", name="prod")
                a_in = at[0:half, :].rearrange("p (i k) -> p i k", k=NT) \
                    .unsqueeze(2).broadcast_to([half, NT, NT, NT])
                b_in = bt[0:half, :].rearrange("p (k j) -> p j k", j=NT) \
                    .unsqueeze(1).broadcast_to([half, NT, NT, NT])
                pv = prod[0:half, :].rearrange("p (i j k) -> p i j k", j=NT, k=NT)
                nc.vector.tensor_tensor(out=pv, in0=a_in, in1=b_in, op=ALU.mult)
                nxt = pCt.tile([128, NT * NT], F32, tag="nxt", name="nxt")
                nc.vector.tensor_reduce(out=nxt[0:half, :], in_=pv, axis=AX.X,
                                        op=ALU.add)
                mx2 = pCt.tile([128, 1], F32, tag="mx2", name="mx2")
                rc2 = pCt.tile([128, 1], F32, tag="rc2", name="rc2")
                nc.vector.tensor_reduce(out=mx2[0:half, :], in_=nxt[0:half, :],
                                        axis=AX.X, op=ALU.max)
                nc.vector.reciprocal(rc2[0:half, :], mx2[0:half, :])
                nc.vector.tensor_scalar_mul(nxt[0:half, :], nxt[0:half, :],
                                            rc2[0:half, 0:1])
                nlg = pCt.tile([128, 1], F32, tag="nlg", name="nlg")
                nc.scalar.activation(nlg[0:half, :], mx2[0:half, :], AF.Ln)
                nc.vector.tensor_tensor(out=nlg[0:half, :], in0=nlg[0:half, :],
                                        in1=alg[0:half, :], op=ALU.add)
                nc.vector.tensor_tensor(out=nlg[0:half, :], in0=nlg[0:half, :],
                                        in1=blg[0:half, :], op=ALU.add)
                cur, curlg, nslots = nxt, nlg, half

            # denom_b = ln(sum_j cur[b, j] * exp(end_j)) + lognorm - S*LNS
            pe3 = pone()
            nc.tensor.transpose(pe3[:, 0:NT], en9[:, 0:1], ident_sb[0:NT, 0:NT])
            enF = pCt.tile([1, NT], F32, tag="enF", name="enF")
            nc.scalar.activation(enF[:], pe3[:, 0:NT], AF.Exp)
            enR = pCt.tile([BL, NT], F32, tag="enR", name="enR")
            nc.gpsimd.partition_broadcast(enR[:], enF[0:1, :])
            dtmp = pCt.tile([BL, NT], F32, tag="dtmp", name="dtmp")
            nc.vector.tensor_tensor(out=dtmp[:], in0=cur[0:BL, 0:NT], in1=enR[:],
                                    op=ALU.mult)
            dot = pCt.tile([BL, 1], F32, tag="dot", name="dot")
            nc.vector.tensor_reduce(out=dot[:], in_=dtmp[:], axis=AX.X,
                                    op=ALU.add)
            den = pCt.tile([BL, 1], F32, tag="den", name="den")
            nc.scalar.activation(den[:], dot[:], AF.Ln)
            nc.vector.tensor_tensor(out=den[:], in0=den[:], in1=curlg[0:BL, :],
                                    op=ALU.add)
            nc.vector.tensor_scalar_add(den[:], den[:], float(-S * LNS))
            pden = pone()
            nc.tensor.transpose(pden[:, 0:BL], den[:, 0:1],
                                ident_sb[0:BL, 0:BL])
            out_sb = pCt.tile([1, BL], F32, tag="out", name="out")
            nc.vector.tensor_tensor(out=out_sb[:], in0=score[:],
                                    in1=pden[:, 0:BL], op=ALU.subtract)
            nc.sync.dma_start(llhD[:], out_sb[:])

        pers_cm.__exit__(None, None, None)

    nc.compile()
    return nc


# ---------------------------------------------------------------------------
# host-side wrapper
# ---------------------------------------------------------------------------

_CACHE = {}


def _get_nc(S, BL):
    key = (S, BL)
    if key not in _CACHE:
        _CACHE[key] = build(S, BL)
    return _CACHE[key]


def _gate_reorder(wT):
    """[.., 4*HD] with gate blocks (i,f,g,o) -> (i,f,o,g)."""
    i, f, g, o = (wT[..., k * HD:(k + 1) * HD] for k in range(4))
    return np.concatenate([i, f, o, g], axis=-1)


def prep_core_inputs(inputs, S, BL, core):
    inp = {k: np.asarray(v) for k, v in inputs.items()}
    b0 = core * BL
    words = inp["words"][b0:b0 + BL, :S].astype(np.int32)     # [BL, S]
    tags = inp["tags"][b0:b0 + BL, :S].astype(np.float32)
    widx = np.ascontiguousarray(words.T).reshape(S * BL, 1)
    tagsFv = np.ascontiguousarray(tags.T).reshape(1, S * BL)

    d = {
        "emb": np.ascontiguousarray(inp["emb_table"].astype(np.float32)),
        "widx": widx,
        "tagsF": tagsFv.astype(np.float32),
        "woT": np.ascontiguousarray(inp["W_out"].T.astype(np.float32)),
        "bout": inp["b_out"].astype(np.float32).reshape(NT, 1),
        "trans": inp["trans"].astype(np.float32),
        "transT": np.ascontiguousarray(inp["trans"].T.astype(np.float32)),
        "startadj": (inp["start_trans"].astype(np.float32) + LNS).reshape(NT, 1),
        "start9": inp["start_trans"].astype(np.float32).reshape(NT, 1),
        "end9": inp["end_trans"].astype(np.float32).reshape(NT, 1),
        "iota9": np.arange(NT, dtype=np.float32).reshape(NT, 1),
        "ones9": np.ones((NT, 1), np.float32),
        "ident": np.eye(128, dtype=np.float32),
    }
    for dd, suf in (("f", "_f"), ("b", "_b")):
        wih = inp["Wih" + suf].astype(np.float32)            # [4HD, E]
        whh = inp["Whh" + suf].astype(np.float32)            # [4HD, HD]
        wihTv = np.zeros((EP, NG * HD), np.float32)
        wihTv[:E, :] = wih.T
        wihR = _gate_reorder(wihTv)
        whhR = _gate_reorder(np.ascontiguousarray(whh.T))
        bias = (inp["bih" + suf] + inp["bhh" + suf]).astype(np.float32)
        biasR = np.ascontiguousarray(_gate_reorder(bias[None, :])[0]
                                     .reshape(NG, HD).T)
        # pre-halve i,f,o so sigmoid(x) = 0.5*tanh(x/2)+0.5 needs no scaling
        wihR[:, 0:3 * HD] *= 0.5
        whhR[:, 0:3 * HD] *= 0.5
        biasR[:, 0:3] *= 0.5
        d[f"wihT_{dd}"] = wihR
        d[f"whhT_{dd}"] = whhR
        d[f"bias_{dd}"] = biasR

    SLOTW = BL * NT
    mt = np.zeros((NT, CH * SLOTW), np.float32)
    for c in range(1, CH):
        for b in range(BL):
            s = c * BL + b
            for j in range(NT):
                mt[j, s * NT + j] = 1.0
    d["mtinit"] = mt
    return d


def _run(inputs, S=512, BL=8, trace=False, **kw):
    nc = _get_nc(S, BL)
    in_maps = [prep_core_inputs(inputs, S, BL, c) for c in range(NCORES)]
    res = run_bass_kernel_spmd(nc, in_maps, core_ids=list(range(NCORES)),
                               trace=trace, **kw)
    llh = np.concatenate([res.results[c]["llh"].reshape(BL)
                          for c in range(NCORES)])
    return llh, res


def kernel(**inputs) -> np.ndarray:
    llh, _ = _run(inputs, S=512, BL=8)
    return np.float32(-(llh.mean()))


# revision 19
# speedup vs baseline: 1.0862x; 1.0862x over previous
"""Trainium2 Bass kernel: BiLSTM + CRF negative log-likelihood (mean over batch).

Contract: kernel(**inputs) takes the FULL unsharded inputs (B=64, S=512) and
returns the scalar fp32 NLL.  Internally the batch is sharded 8 ways across
8 NeuronCores (8 sequences per core); the embedding table is replicated and
gathered on-device via indirect DMA.  Each core computes the per-sequence
log-likelihood for its 8 sequences; the host averages the 64 values.

Mask is assumed all-ones (as produced by the problem's setup_inputs).

Per-core layout choices:
 - token column index = t*BL + b (t-major), BL = 8 sequences per core
 - LSTM state feature-on-partition: h, c are [128, BL]
 - gate order re-packed (i,f,o,g) so one sigmoid covers i,f,o
 - CRF denominator: exp-space chunked parallel scan over 16 chunks
   (slots (chunk,b) = 128 partitions in the combine stage), with the 9x9
   exp(trans) as the PE stationary during the scan.
"""
import numpy as np

import concourse.bacc as bacc
import concourse.bass as bass
import concourse.mybir as mybir
import concourse.tile as tile
from concourse.bass_utils import run_bass_kernel_spmd

AF = mybir.ActivationFunctionType
ALU = mybir.AluOpType
AX = mybir.AxisListType
F32 = mybir.dt.float32
BF16 = mybir.dt.bfloat16
I32 = mybir.dt.int32

V, E, EP = 100000, 300, 384
HD, NG = 128, 4
NT = 9
NCORES = 8
CH = 16
LNS = -2.0

DIRS = ("f", "b")


def build(S, BL):
    NTOK = S * BL
    TPT = 128 // BL
    NTT = NTOK // 128
    CL = S // CH
    GW = NG * BL                 # 32
    SLOTW = BL * NT              # 72

    nc = bacc.Bacc(None, target_bir_lowering=False, debug=False)

    emb = nc.dram_tensor("emb", [V, E], F32, kind="ExternalInput")
    widx = nc.dram_tensor("widx", [NTOK, 1], I32, kind="ExternalInput")
    tagsF = nc.dram_tensor("tagsF", [1, NTOK], F32, kind="ExternalInput")
    wihT = {d: nc.dram_tensor(f"wihT_{d}", [EP, NG * HD], F32, kind="ExternalInput")
            for d in DIRS}
    whhT = {d: nc.dram_tensor(f"whhT_{d}", [HD, NG * HD], F32, kind="ExternalInput")
            for d in DIRS}
    biasD = {d: nc.dram_tensor(f"bias_{d}", [HD, NG], F32, kind="ExternalInput")
             for d in DIRS}
    woT = nc.dram_tensor("woT", [2 * HD, NT], F32, kind="ExternalInput")
    bout = nc.dram_tensor("bout", [NT, 1], F32, kind="ExternalInput")
    transD = nc.dram_tensor("trans", [NT, NT], F32, kind="ExternalInput")
    transTD = nc.dram_tensor("transT", [NT, NT], F32, kind="ExternalInput")
    startAdjD = nc.dram_tensor("startadj", [NT, 1], F32, kind="ExternalInput")
    start9D = nc.dram_tensor("start9", [NT, 1], F32, kind="ExternalInput")
    end9D = nc.dram_tensor("end9", [NT, 1], F32, kind="ExternalInput")
    iotaD = nc.dram_tensor("iota9", [NT, 1], F32, kind="ExternalInput")
    ones9D = nc.dram_tensor("ones9", [NT, 1], F32, kind="ExternalInput")
    identD = nc.dram_tensor("ident", [128, 128], F32, kind="ExternalInput")
    mtinitD = nc.dram_tensor("mtinit", [NT, CH * SLOTW], F32, kind="ExternalInput")
    llhD = nc.dram_tensor("llh", [1, BL], F32, kind="ExternalOutput")

    with tile.TileContext(nc) as tc:
        # ---------------- persistent tiles ----------------
        pers_cm = tc.tile_pool(name="pers", bufs=1)
        pers = pers_cm.__enter__()
        H = {d: pers.tile([128, NTOK], BF16, tag=f"H{d}", name=f"H{d}") for d in DIRS}
        whh_sb = {}
        bias_sb = {}
        for d in DIRS:
            whhf = pers.tile([HD, NG * HD], F32, tag=f"whhf{d}", name=f"whhf{d}")
            nc.sync.dma_start(whhf[:], whhT[d][:])
            whh_sb[d] = pers.tile([HD, NG * HD], BF16, tag=f"whh{d}", name=f"whh{d}")
            nc.vector.tensor_copy(out=whh_sb[d][:], in_=whhf[:])
            bias_sb[d] = pers.tile([HD, NG], F32, tag=f"bias{d}", name=f"bias{d}")
            nc.sync.dma_start(bias_sb[d][:], biasD[d][:])
        ident_sb = pers.tile([128, 128], F32, tag="ident", name="ident")
        nc.sync.dma_start(ident_sb[:], identD[:])
        ident_bf = pers.tile([128, 128], BF16, tag="identbf", name="identbf")
        nc.vector.tensor_copy(out=ident_bf[:], in_=ident_sb[:])
        c_st = {d: pers.tile([128, BL], F32, tag=f"c{d}", name=f"c{d}") for d in DIRS}
        z8 = pers.tile([128, BL], BF16, tag="z8", name="z8")
        nc.vector.memset(z8[:], 0.0)
        for d in DIRS:
            nc.vector.memset(c_st[d][:], 0.0)

        # ---------------- input projections into Gin ----------------
        ging_cm = tc.tile_pool(name="gin", bufs=1)
        ging = ging_cm.__enter__()
        gin = {d: ging.tile([128, S * GW], BF16, tag=f"gin{d}", name=f"gin{d}") for d in DIRS}
        ginv = {d: gin[d][:].rearrange("p (t x) -> p t x", x=GW) for d in DIRS}

        with (
            tc.tile_pool(name="pA", bufs=3) as pA,
            tc.tile_pool(name="pAw", bufs=1) as pAw,
            tc.tile_pool(name="ppA", bufs=2, space="PSUM") as ppA,
            tc.tile_pool(name="pB", bufs=3) as pB,
            tc.tile_pool(name="ppB", bufs=2, space="PSUM") as ppB,
        ):
            wih_sb = {d: [] for d in DIRS}
            for d in DIRS:
                for k in range(3):
                    wtf = pAw.tile([128, NG * HD], F32, tag=f"wihf{d}{k}", name=f"wihf{d}{k}")
                    nc.sync.dma_start(wtf[:], wihT[d][k * 128:(k + 1) * 128, :])
                    wt = pAw.tile([128, NG * HD], BF16, tag=f"wih{d}{k}", name=f"wih{d}{k}")
                    nc.vector.tensor_copy(out=wt[:], in_=wtf[:])
                    wih_sb[d].append(wt)
            tporder = []
            for i in range((NTT + 1) // 2):
                tporder.append(i)
                if NTT - 1 - i > i:
                    tporder.append(NTT - 1 - i)
            for tp in tporder:
                idx = pA.tile([128, 1], I32, tag="idx", name="idx")
                nc.sync.dma_start(idx[:], widx[tp * 128:(tp + 1) * 128, :])
                xg = pA.tile([128, EP], F32, tag="xg", name="xg")
                nc.vector.memset(xg[:, E:EP], 0.0)
                nc.gpsimd.indirect_dma_start(
                    out=xg[:, 0:E], out_offset=None, in_=emb[:],
                    in_offset=bass.IndirectOffsetOnAxis(ap=idx[:, 0:1], axis=0),
                )
                xt = []
                for k in range(3):
                    pt = ppA.tile([128, 128], F32, tag="pt", name="pt")
                    nc.tensor.transpose(pt[:], xg[:, k * 128:(k + 1) * 128],
                                        ident_sb[:])
                    xk = pA.tile([128, 128], BF16, tag=f"xt{k}", name=f"xt{k}")
                    nc.vector.tensor_copy(out=xk[:], in_=pt[:])
                    xt.append(xk)
                for d in DIRS:
                    for g in range(NG):
                        ps = ppA.tile([128, 128], F32, tag="ps", name="ps")
                        for k in range(3):
                            nc.tensor.matmul(
                                ps[:], lhsT=wih_sb[d][k][:, g * 128:(g + 1) * 128],
                                rhs=xt[k][:], start=(k == 0), stop=(k == 2))
                        dst = ginv[d][:, tp * TPT:(tp + 1) * TPT,
                                      g * BL:(g + 1) * BL]
                        src = ps[:].rearrange("p (t b) -> p t b", b=BL)
                        nc.vector.tensor_scalar_add(dst, src,
                                                    bias_sb[d][:, g:g + 1])

            # ---------------- BiLSTM recurrence ----------------
            # i,f,o weight blocks are pre-halved host-side, so one tanh
            # covers all gates: sigmoid(x) = 0.5*tanh(x/2) + 0.5.
            # phase-sorted emission: both directions' same-phase ops are
            # adjacent in each engine's FIFO, so neither chain head-of-line
            # blocks the other.
            def lstm_pair(tf, tb, hf, hb):
                tt = {"f": tf, "b": tb}
                hh = {"f": hf, "b": hb}
                ps, T, sg, t1, c2, tc2 = {}, {}, {}, {}, {}, {}
                for d in DIRS:
                    ps[d] = ppB.tile([128, GW], F32, tag=f"ps{d}", name=f"ps{d}")
                    nc.tensor.matmul(ps[d][:], lhsT=ident_bf[:],
                                     rhs=gin[d][:, tt[d] * GW:(tt[d] + 1) * GW],
                                     start=True, stop=False)
                    for g in range(NG):
                        nc.tensor.matmul(ps[d][:, g * BL:(g + 1) * BL],
                                         lhsT=whh_sb[d][:, g * 128:(g + 1) * 128],
                                         rhs=hh[d], start=False,
                                         stop=(g == NG - 1))
                for d in DIRS:
                    T[d] = pB.tile([128, GW], F32, tag=f"T{d}", name=f"T{d}")
                    nc.scalar.activation(T[d][:], ps[d][:], AF.Tanh)
                for d in DIRS:
                    sg[d] = pB.tile([128, 3 * BL], F32, tag=f"sg{d}", name=f"sg{d}")
                    nc.vector.tensor_scalar(out=sg[d][:], in0=T[d][:, 0:3 * BL],
                                            scalar1=0.5, scalar2=0.5,
                                            op0=ALU.mult, op1=ALU.add)
                for d in DIRS:
                    t1[d] = pB.tile([128, BL], F32, tag=f"t1{d}", name=f"t1{d}")
                    nc.vector.tensor_tensor(out=t1[d][:], in0=sg[d][:, 0:BL],
                                            in1=T[d][:, 3 * BL:GW], op=ALU.mult)
                for d in DIRS:
                    c2[d] = pB.tile([128, BL], F32, tag=f"c2{d}", name=f"c2{d}")
                    nc.vector.tensor_tensor(out=c2[d][:], in0=sg[d][:, BL:2 * BL],
                                            in1=c_st[d][:], op=ALU.mult)
                for d in DIRS:
                    nc.vector.tensor_tensor(out=c_st[d][:], in0=c2[d][:],
                                            in1=t1[d][:], op=ALU.add)
                for d in DIRS:
                    tc2[d] = pB.tile([128, BL], F32, tag=f"tc{d}", name=f"tc{d}")
                    nc.scalar.activation(tc2[d][:], c_st[d][:], AF.Tanh)
                for d in DIRS:
                    nc.vector.tensor_tensor(
                        out=H[d][:, tt[d] * BL:(tt[d] + 1) * BL],
                        in0=sg[d][:, 2 * BL:3 * BL], in1=tc2[d][:], op=ALU.mult)

            for step in range(S):
                tf = step
                tb = S - 1 - step
                hf = z8[:] if step == 0 else H["f"][:, (tf - 1) * BL:tf * BL]
                hb = z8[:] if step == 0 else H["b"][:, (tb + 1) * BL:(tb + 2) * BL]
                lstm_pair(tf, tb, hf, hb)

        ging_cm.__exit__(None, None, None)

        # ---------------- emissions + CRF ----------------
        with (
            tc.tile_pool(name="pC", bufs=1) as pC,
            tc.tile_pool(name="pCt", bufs=2) as pCt,
            tc.tile_pool(name="ppC", bufs=2, space="PSUM") as ppC,
            tc.tile_pool(name="ppD", bufs=2, space="PSUM") as ppD,
            tc.tile_pool(name="ppE", bufs=3, space="PSUM") as ppE,
        ):
            def pbig():          # [NT, 512] psum tiles (emissions/numerator/scan)
                return ppC.tile([NT, 512], F32, tag="pbig", name="pbig")

            def ptr():           # [128, NT] transpose psum tiles
                return ppD.tile([128, NT], BF16, tag="ptr", name="ptr")

            def pone():          # [1, <=NT] psum tiles
                return ppE.tile([1, NT], F32, tag="pone", name="pone")

            wo0f = pC.tile([128, NT], F32, tag="wo0f", name="wo0f")
            wo1f = pC.tile([128, NT], F32, tag="wo1f", name="wo1f")
            nc.sync.dma_start(wo0f[:], woT[0:128, :])
            nc.sync.dma_start(wo1f[:], woT[128:256, :])
            wo0 = pC.tile([128, NT], BF16, tag="wo0", name="wo0")
            wo1 = pC.tile([128, NT], BF16, tag="wo1", name="wo1")
            nc.vector.tensor_copy(out=wo0[:], in_=wo0f[:])
            nc.vector.tensor_copy(out=wo1[:], in_=wo1f[:])
            bout_sb = pC.tile([NT, 1], F32, tag="bout", name="bout")
            nc.sync.dma_start(bout_sb[:], bout[:])
            em = pC.tile([NT, NTOK], F32, tag="em", name="em")
            NCHK = (NTOK + 511) // 512
            for n in range(NCHK):
                lo, hi = n * 512, min((n + 1) * 512, NTOK)
                pe = pbig()
                nc.tensor.matmul(pe[:, 0:hi - lo], lhsT=wo0[:],
                                 rhs=H["f"][:, lo:hi], start=True, stop=False)
                nc.tensor.matmul(pe[:, 0:hi - lo], lhsT=wo1[:],
                                 rhs=H["b"][:, lo:hi], start=False, stop=True)
                nc.scalar.activation(em[:, lo:hi], pe[:, 0:hi - lo], AF.Identity,
                                     bias=bout_sb[:, 0:1])

            # --- numerator (gold path score) ---
            trT = pC.tile([NT, NT], F32, tag="trT", name="trT")
            nc.sync.dma_start(trT[:], transTD[:])
            st9 = pC.tile([NT, 1], F32, tag="st9", name="st9")
            nc.sync.dma_start(st9[:], start9D[:])
            en9 = pC.tile([NT, 1], F32, tag="en9", name="en9")
            nc.sync.dma_start(en9[:], end9D[:])
            io9 = pC.tile([NT, 1], F32, tag="io9", name="io9")
            nc.sync.dma_start(io9[:], iotaD[:])
            on9 = pC.tile([NT, 1], F32, tag="on9", name="on9")
            nc.sync.dma_start(on9[:], ones9D[:])

            tagR = pC.tile([NT, NTOK], F32, tag="tagR", name="tagR")
            tg1 = pC.tile([1, NTOK], F32, tag="tg1", name="tg1")
            nc.sync.dma_start(tg1[:], tagsF[:])
            nc.gpsimd.partition_broadcast(tagR[:], tg1[0:1, :])
            oh = pC.tile([NT, NTOK], F32, tag="oh", name="oh")
            nc.vector.tensor_tensor(out=oh[:],
                                    in0=io9[:, 0:1].to_broadcast([NT, NTOK]),
                                    in1=tagR[:], op=ALU.is_equal)
            t1f = pC.tile([NT, NTOK], F32, tag="t1f", name="t1f")
            for n in range(NCHK):
                lo, hi = n * 512, min((n + 1) * 512, NTOK)
                pn = pbig()
                nc.tensor.matmul(pn[:, 0:hi - lo], lhsT=trT[:], rhs=oh[:, lo:hi],
                                 start=True, stop=True)
                nc.vector.tensor_copy(out=t1f[:, lo:hi], in_=pn[:, 0:hi - lo])

            tmp = pC.tile([NT, NTOK], F32, tag="tmp", name="tmp")
            nc.vector.tensor_tensor(out=tmp[:], in0=em[:], in1=oh[:], op=ALU.mult)
            sc = pCt.tile([NT, BL], F32, tag="sc_em", name="sc_em")
            nc.vector.tensor_reduce(
                out=sc[:], in_=tmp[:].rearrange("p (t b) -> p b t", b=BL),
                axis=AX.X, op=ALU.add)
            tmq = pC.tile([NT, NTOK - BL], F32, tag="tmq", name="tmq")
            nc.vector.tensor_tensor(out=tmq[:], in0=oh[:, 0:NTOK - BL],
                                    in1=t1f[:, BL:NTOK], op=ALU.mult)
            sctr = pCt.tile([NT, BL], F32, tag="sc_tr", name="sc_tr")
            nc.vector.tensor_reduce(
                out=sctr[:], in_=tmq[:].rearrange("p (t b) -> p b t", b=BL),
                axis=AX.X, op=ALU.add)
            scse = pCt.tile([NT, BL], F32, tag="sc_se", name="sc_se")
            nc.vector.tensor_tensor(out=scse[:],
                                    in0=st9[:, 0:1].to_broadcast([NT, BL]),
                                    in1=oh[:, 0:BL], op=ALU.mult)
            scee = pCt.tile([NT, BL], F32, tag="sc_ee", name="sc_ee")
            nc.vector.tensor_tensor(out=scee[:],
                                    in0=en9[:, 0:1].to_broadcast([NT, BL]),
                                    in1=oh[:, NTOK - BL:NTOK], op=ALU.mult)
            nc.vector.tensor_tensor(out=sc[:], in0=sc[:], in1=sctr[:], op=ALU.add)
            nc.vector.tensor_tensor(out=scse[:], in0=scse[:], in1=scee[:],
                                    op=ALU.add)
            nc.vector.tensor_tensor(out=sc[:], in0=sc[:], in1=scse[:], op=ALU.add)
            psS = pone()
            nc.tensor.matmul(psS[:, 0:BL], lhsT=on9[:, 0:1], rhs=sc[:],
                             start=True, stop=True)
            score = pCt.tile([1, BL], F32, tag="score", name="score")
            nc.vector.tensor_copy(out=score[:], in_=psS[:, 0:BL])

            # --- denominator: exp-space chunked parallel scan ---
            trS = pC.tile([NT, NT], F32, tag="trS", name="trS")
            nc.sync.dma_start(trS[:], transD[:])
            Emat = pC.tile([NT, NT], BF16, tag="Emat", name="Emat")
            nc.scalar.activation(Emat[:], trS[:], AF.Exp)
            lnsC = pCt.tile([NT, 1], F32, tag="lnsC", name="lnsC")
            nc.vector.memset(lnsC[:], float(LNS))
            wem = pC.tile([NT, NTOK], F32, tag="wem", name="wem")
            nc.scalar.activation(wem[:], em[:], AF.Exp, bias=lnsC[:, 0:1])
            wv = wem[:].rearrange("p (c u b) -> p c u b", u=CL, b=BL)
            stA = pC.tile([NT, 1], F32, tag="stA", name="stA")
            nc.sync.dma_start(stA[:], startAdjD[:])
            Mt = pC.tile([NT, CH * SLOTW], BF16, tag="Mt", name="Mt")
            nc.gpsimd.dma_start(Mt[:], mtinitD[:])
            tmp0 = pCt.tile([NT, BL], F32, tag="tmp0", name="tmp0")
            nc.scalar.activation(tmp0[:], em[:, 0:BL], AF.Exp, bias=stA[:, 0:1])
            mtv = Mt[:].rearrange("p (s i) -> p s i", i=NT)
            for i in range(NT):
                nc.vector.tensor_copy(
                    out=mtv[:, 0:BL, i:i + 1],
                    in_=tmp0[:].rearrange("p (b o) -> p b o", o=1))

            R = [(0, 5), (5, 10), (10, CH)]
            for u in range(CL):
                for (c0, c1) in R:
                    cc0 = 1 if (u == 0 and c0 == 0) else c0
                    if cc0 >= c1:
                        continue
                    nch = c1 - cc0
                    lo, hi = cc0 * SLOTW, c1 * SLOTW
                    pm = pbig()
                    nc.tensor.matmul(pm[:, 0:hi - lo], lhsT=Emat[:],
                                     rhs=Mt[:, lo:hi], start=True, stop=True)
                    dst = Mt[:].rearrange("p (c b i) -> p c b i", b=BL, i=NT)[
                        :, cc0:c1, :, :]
                    src = pm[:, 0:hi - lo].rearrange("p (c b i) -> p c b i",
                                                     b=BL, i=NT)
                    w_in = wv[:, cc0:c1, u:u + 1, :].squeeze(2).unsqueeze(3) \
                        .broadcast_to([NT, nch, BL, NT])
                    nc.vector.tensor_tensor(out=dst, in0=src, in1=w_in,
                                            op=ALU.mult)

            # transpose Mt -> ct [slot, i*NT+j]
            ct = pC.tile([128, NT * NT], F32, tag="ct", name="ct")
            id9b = pCt.tile([NT, NT], BF16, tag="id9b", name="id9b")
            nc.vector.tensor_copy(out=id9b[:], in_=ident_sb[0:NT, 0:NT])
            mtt = Mt[:].rearrange("p (s i) -> p s i", i=NT)
            for i in range(NT):
                pt2 = ptr()
                nc.tensor.transpose(pt2[:], mtt[:, :, i:i + 1].squeeze(2),
                                    id9b[:])
                nc.vector.tensor_copy(out=ct[:, i * NT:(i + 1) * NT], in_=pt2[:])

            lgn = pC.tile([128, 1], F32, tag="lgn", name="lgn")
            mx = pCt.tile([128, 1], F32, tag="mx", name="mx")
            rc = pCt.tile([128, 1], F32, tag="rc", name="rc")
            nc.vector.tensor_reduce(out=mx[:], in_=ct[:], axis=AX.X, op=ALU.max)
            nc.vector.reciprocal(rc[:], mx[:])
            nc.vector.tensor_scalar_mul(ct[:], ct[:], rc[:, 0:1])
            nc.scalar.activation(lgn[:], mx[:], AF.Ln)

            cur, curlg, nslots = ct, lgn, 128
            for lvl in range(4):
                half = nslots // 2
                at = pCt.tile([128, NT * NT], F32, tag="at", name="at")
                bt = pCt.tile([128, NT * NT], F32, tag="bt", name="bt")
                alg = pCt.tile([128, 1], F32, tag="alg", name="alg")
                blg = pCt.tile([128, 1], F32, tag="blg", name="blg")
                for q in range(half // BL):
                    e0, e1 = (2 * q) * BL, (2 * q + 1) * BL
                    nc.sync.dma_start(at[q * BL:(q + 1) * BL, :],
                                      cur[e0:e0 + BL, :])
                    nc.sync.dma_start(bt[q * BL:(q + 1) * BL, :],
                                      cur[e1:e1 + BL, :])
                    nc.sync.dma_start(alg[q * BL:(q + 1) * BL, :],
                                      curlg[e0:e0 + BL, :])
                    nc.sync.dma_start(blg[q * BL:(q + 1) * BL, :],
                                      curlg[e1:e1 + BL, :])
                prod = pCt.tile([128, NT * NT * NT], F32, tag="prod", name="prod")
                a_in = at[0:half, :].rearrange("p (i k) -> p i k", k=NT) \
                    .unsqueeze(2).broadcast_to([half, NT, NT, NT])
                b_in = bt[0:half, :].rearrange("p (k j) -> p j k", j=NT) \
                    .unsqueeze(1).broadcast_to([half, NT, NT, NT])
                pv = prod[0:half, :].rearrange("p (i j k) -> p i j k", j=NT, k=NT)
                nc.vector.tensor_tensor(out=pv, in0=a_in, in1=b_in, op=ALU.mult)
                nxt = pCt.tile([128, NT * NT], F32, tag="nxt", name="nxt")
                nc.vector.tensor_reduce(out=nxt[0:half, :], in_=pv, axis=AX.X,
                                        op=ALU.add)
                mx2 = pCt.tile([128, 1], F32, tag="mx2", name="mx2")
                rc2 = pCt.tile([128, 1], F32, tag="rc2", name="rc2")
                nc.vector.tensor_reduce(out=mx2[0:half, :], in_=nxt[0:half, :],
                                        axis=AX.X, op=ALU.max)
                nc.vector.reciprocal(rc2[0:half, :], mx2[0:half, :])
                nc.vector.tensor_scalar_mul(nxt[0:half, :], nxt[0:half, :],
                                            rc2[0:half, 0:1])
                nlg = pCt.tile([128, 1], F32, tag="nlg", name="nlg")
                nc.scalar.activation(nlg[0:half, :], mx2[0:half, :], AF.Ln)
                nc.vector.tensor_tensor(out=nlg[0:half, :], in0=nlg[0:half, :],
                                        in1=alg[0:half, :], op=ALU.add)
                nc.vector.tensor_tensor(out=nlg[0:half, :], in0=nlg[0:half, :],
                                        in1=blg[0:half, :], op=ALU.add)
                cur, curlg, nslots = nxt, nlg, half

            # denom_b = ln(sum_j cur[b, j] * exp(end_j)) + lognorm - S*LNS
            pe3 = pone()
            nc.tensor.transpose(pe3[:, 0:NT], en9[:, 0:1], ident_sb[0:NT, 0:NT])
            enF = pCt.tile([1, NT], F32, tag="enF", name="enF")
            nc.scalar.activation(enF[:], pe3[:, 0:NT], AF.Exp)
            enR = pCt.tile([BL, NT], F32, tag="enR", name="enR")
            nc.gpsimd.partition_broadcast(enR[:], enF[0:1, :])
            dtmp = pCt.tile([BL, NT], F32, tag="dtmp", name="dtmp")
            nc.vector.tensor_tensor(out=dtmp[:], in0=cur[0:BL, 0:NT], in1=enR[:],
                                    op=ALU.mult)
            dot = pCt.tile([BL, 1], F32, tag="dot", name="dot")
            nc.vector.tensor_reduce(out=dot[:], in_=dtmp[:], axis=AX.X,
                                    op=ALU.add)
            den = pCt.tile([BL, 1], F32, tag="den", name="den")
            nc.scalar.activation(den[:], dot[:], AF.Ln)
            nc.vector.tensor_tensor(out=den[:], in0=den[:], in1=curlg[0:BL, :],
                                    op=ALU.add)
            nc.vector.tensor_scalar_add(den[:], den[:], float(-S * LNS))
            pden = pone()
            nc.tensor.transpose(pden[:, 0:BL], den[:, 0:1],
                                ident_sb[0:BL, 0:BL])
            out_sb = pCt.tile([1, BL], F32, tag="out", name="out")
            nc.vector.tensor_tensor(out=out_sb[:], in0=score[:],
                                    in1=pden[:, 0:BL], op=ALU.subtract)
            nc.sync.dma_start(llhD[:], out_sb[:])

        pers_cm.__exit__(None, None, None)

    nc.compile()
    return nc


# ---------------------------------------------------------------------------
# host-side wrapper
# ---------------------------------------------------------------------------

_CACHE = {}


def _get_nc(S, BL):
    key = (S, BL)
    if key not in _CACHE:
        _CACHE[key] = build(S, BL)
    return _CACHE[key]


def _gate_reorder(wT):
    """[.., 4*HD] with gate blocks (i,f,g,o) -> (i,f,o,g)."""
    i, f, g, o = (wT[..., k * HD:(k + 1) * HD] for k in range(4))
    return np.concatenate([i, f, o, g], axis=-1)


def prep_core_inputs(inputs, S, BL, core):
    inp = {k: np.asarray(v) for k, v in inputs.items()}
    b0 = core * BL
    words = inp["words"][b0:b0 + BL, :S].astype(np.int32)     # [BL, S]
    tags = inp["tags"][b0:b0 + BL, :S].astype(np.float32)
    widx = np.ascontiguousarray(words.T).reshape(S * BL, 1)
    tagsFv = np.ascontiguousarray(tags.T).reshape(1, S * BL)

    d = {
        "emb": np.ascontiguousarray(inp["emb_table"].astype(np.float32)),
        "widx": widx,
        "tagsF": tagsFv.astype(np.float32),
        "woT": np.ascontiguousarray(inp["W_out"].T.astype(np.float32)),
        "bout": inp["b_out"].astype(np.float32).reshape(NT, 1),
        "trans": inp["trans"].astype(np.float32),
        "transT": np.ascontiguousarray(inp["trans"].T.astype(np.float32)),
        "startadj": (inp["start_trans"].astype(np.float32) + LNS).reshape(NT, 1),
        "start9": inp["start_trans"].astype(np.float32).reshape(NT, 1),
        "end9": inp["end_trans"].astype(np.float32).reshape(NT, 1),
        "iota9": np.arange(NT, dtype=np.float32).reshape(NT, 1),
        "ones9": np.ones((NT, 1), np.float32),
        "ident": np.eye(128, dtype=np.float32),
    }
    for dd, suf in (("f", "_f"), ("b", "_b")):
        wih = inp["Wih" + suf].astype(np.float32)            # [4HD, E]
        whh = inp["Whh" + suf].astype(np.float32)            # [4HD, HD]
        wihTv = np.zeros((EP, NG * HD), np.float32)
        wihTv[:E, :] = wih.T
        wihR = _gate_reorder(wihTv)
        whhR = _gate_reorder(np.ascontiguousarray(whh.T))
        bias = (inp["bih" + suf] + inp["bhh" + suf]).astype(np.float32)
        biasR = np.ascontiguousarray(_gate_reorder(bias[None, :])[0]
                                     .reshape(NG, HD).T)
        # pre-halve i,f,o so sigmoid(x) = 0.5*tanh(x/2)+0.5 needs no scaling
        wihR[:, 0:3 * HD] *= 0.5
        whhR[:, 0:3 * HD] *= 0.5
        biasR[:, 0:3] *= 0.5
        d[f"wihT_{dd}"] = wihR
        d[f"whhT_{dd}"] = whhR
        d[f"bias_{dd}"] = biasR

    SLOTW = BL * NT
    mt = np.zeros((NT, CH * SLOTW), np.float32)
    for c in range(1, CH):
        for b in range(BL):
            s = c * BL + b
            for j in range(NT):
                mt[j, s * NT + j] = 1.0
    d["mtinit"] = mt
    return d


def _run(inputs, S=512, BL=8, trace=False, **kw):
    nc = _get_nc(S, BL)
    in_maps = [prep_core_inputs(inputs, S, BL, c) for c in range(NCORES)]
    res = run_bass_kernel_spmd(nc, in_maps, core_ids=list(range(NCORES)),
                               trace=trace, **kw)
    llh = np.concatenate([res.results[c]["llh"].reshape(BL)
                          for c in range(NCORES)])
    return llh, res


def kernel(**inputs) -> np.ndarray:
    llh, _ = _run(inputs, S=512, BL=8)
    return np.float32(-(llh.mean()))
